# revision 5
# baseline (speedup 1.0000x reference)
"""Spectral diffusion kernel for Trainium2, 8-core SPMD.

Computes out = evecs @ (exp(-evals*clamp(t,1e-8)) * (evecs.T @ x))
with N=8192, C=256 in fp32 (PE matmuls in fp32r / TF32-like).

Sharding (scheme C):
  pass 1 (k-sharded): core m computes z_m = evecs[:, Km].T @ x exactly
      ([1024, 256]), scales by coefs[Km] -> w_m.
  AllGather w (1 MB/rank -> 8 MB on every core).
  pass 2 (n-sharded): core m computes out[Nm, :] = evecs[Nm, :] @ w
      via PE-transposed evecs tiles, accumulating over k in PSUM.
      No AllReduce anywhere.
"""
import numpy as np

import concourse.bacc as bacc
import concourse.mybir as mybir
import concourse.tile as tile
from concourse import bass_utils
from concourse.masks import make_identity

dt = mybir.dt

N = 8192
C = 256
N_CORES = 8
KS = N // N_CORES     # 1024: per-core spectral (pass 1) / row (pass 2) slice
P = 128
NT = N // P           # 64 n-tiles (pass 1 contraction)
KSUB = KS // P        # 8 k-subtiles per core (pass 1 outputs)
NS = KS // P          # 8 output row-tiles per core (pass 2)
KT = N // P           # 64 k-tiles (pass 2 contraction)
XCH = 8               # x / w resident chunks


def build_program():
    nc = bacc.Bacc(None, target_bir_lowering=False, num_devices=N_CORES)

    a_col = nc.dram_tensor("a_col", [N, KS], dt.float32r, kind="ExternalInput")
    a_row = nc.dram_tensor("a_row", [KS, N], dt.float32r, kind="ExternalInput")
    x_in = nc.dram_tensor("x", [N, C], dt.float32r, kind="ExternalInput")
    ev_in = nc.dram_tensor("ev", [P, KSUB], dt.float32, kind="ExternalInput")
    t_in = nc.dram_tensor("t", [P, 1], dt.float32, kind="ExternalInput")
    out_d = nc.dram_tensor("out", [KS, C], dt.float32, kind="ExternalOutput")

    zin = nc.dram_tensor("zin", [KS, C], dt.float32r, kind="Internal")
    wall = nc.dram_tensor("wall", [N, C], dt.float32r, kind="Internal",
                          addr_space="Shared")

    with tile.TileContext(nc) as tc:
        with tc.tile_pool(name="const", bufs=1) as const:
            # ---- constants: identity (for PE transpose), coefs ----
            ident_f32 = const.tile([P, P], dt.float32)
            make_identity(nc, ident_f32)
            ident = const.tile([P, P], dt.float32r)
            nc.vector.tensor_copy(ident[:], ident_f32[:])

            t_sb = const.tile([P, 1], dt.float32)
            nc.sync.dma_start(t_sb[:], t_in[:])
            nc.vector.tensor_scalar_max(t_sb[:], t_sb[:], 1e-8)
            ev_sb = const.tile([P, KSUB], dt.float32)
            nc.sync.dma_start(ev_sb[:], ev_in[:])
            coef = const.tile([P, KSUB], dt.float32)
            nc.vector.tensor_scalar_mul(coef[:], ev_sb[:], t_sb[:])
            nc.scalar.activation(coef[:], coef[:],
                                 mybir.ActivationFunctionType.Exp, scale=-1.0)

            self_phase1_phase2(nc, tc, a_col, a_row, x_in, out_d, zin, wall,
                               ident, coef)

    nc.compile()
    return nc


def self_phase1_phase2(nc, tc, a_col, a_row, x_in, out_d, zin, wall,
                       ident, coef):
        with (
            tc.tile_pool(name="xpool", bufs=1) as xpool,
            tc.tile_pool(name="apool", bufs=4) as apool,
            tc.tile_pool(name="zps", bufs=1, space="PSUM") as zps,
            tc.tile_pool(name="zsb", bufs=2) as zsb,
        ):
            # ---- pass 1: z = a_col.T @ x  (contract n), k-sharded ----
            x_sb = []
            for i in range(XCH):
                xt = xpool.tile([P, NT // XCH, C], dt.float32r, name=f"xsb{i}")
                nc.sync.dma_start(
                    xt[:],
                    x_in[i * (N // XCH):(i + 1) * (N // XCH), :]
                    .rearrange("(j p) c -> p j c", p=P))
                x_sb.append(xt)

            z_ps = [zps.tile([P, C], dt.float32, name=f"zps{k}")
                    for k in range(KSUB)]

            for nt in range(NT):
                a_t = apool.tile([P, KS], dt.float32r, name="a_t")
                nc.sync.dma_start(a_t[:], a_col[nt * P:(nt + 1) * P, :])
                rhs = x_sb[nt // XCH][:, nt % XCH, :]
                for ks in range(KSUB):
                    nc.tensor.matmul(
                        z_ps[ks][:],
                        a_t[:, ks * P:(ks + 1) * P],
                        rhs,
                        start=(nt == 0), stop=(nt == NT - 1))

            for ks in range(KSUB):
                w_sb = zsb.tile([P, C], dt.float32r, name="w_sb")
                nc.vector.tensor_scalar_mul(
                    w_sb[:], z_ps[ks][:], coef[:, ks:ks + 1])
                nc.sync.dma_start(zin[ks * P:(ks + 1) * P, :], w_sb[:])

            nc.gpsimd.collective_compute(
                "AllGather",
                mybir.AluOpType.bypass,
                replica_groups=[list(range(N_CORES))],
                ins=[zin.ap().opt()],
                outs=[wall.ap().opt()],
            )

        # ---- pass 2: out[Nm,:] = a_row @ w  (contract k), n-sharded ----
        with (
            tc.tile_pool(name="wpool", bufs=1) as wpool,
            tc.tile_pool(name="a2pool", bufs=2) as a2pool,
            tc.tile_pool(name="atpool", bufs=4) as atpool,
            tc.tile_pool(name="tps", bufs=2, space="PSUM") as tps,
            tc.tile_pool(name="ops", bufs=2, space="PSUM") as ops,
            tc.tile_pool(name="osb", bufs=2) as osb,
        ):
            w_sb2 = []
            for i in range(XCH):
                wt = wpool.tile([P, KT // XCH, C], dt.float32r, name=f"wt{i}")
                nc.sync.dma_start(
                    wt[:],
                    wall[i * (N // XCH):(i + 1) * (N // XCH), :]
                    .rearrange("(j p) c -> p j c", p=P))
                w_sb2.append(wt)

            for ns in range(NS):
                a2 = a2pool.tile([P, N], dt.float32r, name="a2")
                for g in range(8):
                    nc.sync.dma_start(
                        a2[:, g * (N // 8):(g + 1) * (N // 8)],
                        a_row[ns * P:(ns + 1) * P,
                              g * (N // 8):(g + 1) * (N // 8)])
                o_ps = ops.tile([P, C], dt.float32, name="o_ps")
                for kt in range(KT):
                    tp = tps.tile([P, P], dt.float32r, name="tp")
                    nc.tensor.transpose(
                        tp[:], a2[:, kt * P:(kt + 1) * P], ident[:])
                    at = atpool.tile([P, P], dt.float32r, name="at")
                    nc.vector.tensor_copy(at[:], tp[:])
                    nc.tensor.matmul(
                        o_ps[:], at[:], w_sb2[kt // XCH][:, kt % XCH, :],
                        start=(kt == 0), stop=(kt == KT - 1))
                o_sb = osb.tile([P, C], dt.float32, name="o_sb")
                nc.vector.tensor_copy(o_sb[:], o_ps[:])
                nc.sync.dma_start(out_d[ns * P:(ns + 1) * P, :], o_sb[:])


def make_in_maps(x, evals, evecs, t):
    x = np.ascontiguousarray(x, dtype=np.float32)
    evals = np.ascontiguousarray(evals, dtype=np.float32)
    evecs = np.ascontiguousarray(evecs, dtype=np.float32)
    t_col = np.full((P, 1), np.float32(t[0]), dtype=np.float32)
    in_maps = []
    for r in range(N_CORES):
        sl = slice(r * KS, (r + 1) * KS)
        in_maps.append({
            "a_col": np.ascontiguousarray(evecs[:, sl]),
            "a_row": np.ascontiguousarray(evecs[sl, :]),
            "x": x,
            "ev": np.ascontiguousarray(evals[sl].reshape(KSUB, P).T),
            "t": t_col,
        })
    return in_maps


_cached = {}


def kernel(x, evals, evecs, t):
    if "nc" not in _cached:
        _cached["nc"] = build_program()
    nc = _cached["nc"]
    in_maps = make_in_maps(x, evals, evecs, t)
    res = bass_utils.run_bass_kernel_spmd(
        nc, in_maps, core_ids=list(range(N_CORES)))
    out = np.concatenate([res.results[r]["out"] for r in range(N_CORES)],
                         axis=0)
    return out.astype(np.float32)
